# revision 19
# baseline (speedup 1.0000x reference)
"""Trainium2 Bass kernel for the RSNN (recurrent spiking NN) problem.

Strategy: model-parallel over the hidden dimension across 8 NeuronCores
(512 hidden units per core), full batch (128) on every core.  Per
timestep each core computes the recurrent matmul for its hidden shard
([128x4096] @ [4096x512]), derives its shard's spikes, and AllGathers
the (PE-transposed) spikes as fp8 so every core has the full spike
vector for the next step.  The input projection (x @ W_fc1) is
pipelined two steps ahead and the readout (spk_acc @ W_out) is computed
per-step from the tau-accumulated spikes - both fill the PE while the
AllGather is in flight.  Partial readout sums (over hidden shards) are
combined on the host.

  o_mem_t = sum_{s<=t} tau^{t-s} spk_s @ W_out = (spk_acc_t) @ W_out
  spk_acc_t = tau * spk_acc_{t-1} + spk_t

The recurrent matmul runs in fp8 DoubleRow mode (2 contraction tiles
per instruction, 2x fp16 throughput): the gathered spikes are exactly
representable in fp8, and W_rec is pre-scaled by 256 (its entries live
in [0, 1/64] - raw e4m3 would put them all in the subnormal range) so
quantization error is ~2^-4 relative.  The spike decision compensates
the scale: psum >= 2560*(0.5 - pre)  <=>  0.1*rec + pre >= 0.5.
One AllGather per step (two half-gathers serialize completely in the
collectives firmware, each paying its own ~7us floor), landed in two
chunks on two DMA queues so the recurrent matmul can start on the
first 16 contraction tiles while the rest lands.
"""

import sys

sys.path.insert(0, "/opt/trn_rl_repo")

import numpy as np

B, T = 128, 64
N_IN, N_HID, N_OUT = 1024, 4096, 1024
TAU = 0.9
THRESH = 0.5
REC_SCALE = 0.1
N_CORES = 8
H_LOC = N_HID // N_CORES  # 512
REC_W_SCALE = 256.0  # fp8 pre-scale for W_rec

_cache = {}


def _build():
    import concourse.bacc as bacc
    import concourse.tile as tile
    from concourse import mybir

    f32 = mybir.dt.float32
    f16 = mybir.dt.float16
    fp8 = mybir.dt.float8e4
    Alu = mybir.AluOpType
    DR = mybir.MatmulPerfMode.DoubleRow

    KT_REC = N_HID // 128  # 32 contraction tiles for the recurrent matmul
    KT_FC1 = N_IN // 128   # 8
    KT_OUT = H_LOC // 128  # 4
    KT_HALF = KT_REC // 2  # 16 k-tiles per landing chunk

    nc = bacc.Bacc(
        "TRN2",
        target_bir_lowering=False,
        debug=False,
        enable_asserts=True,
        num_devices=N_CORES,
    )

    # x transposed+tiled on host: [T, k, p, B] with n_in = 128*k + p
    xt_d = nc.dram_tensor("xt", [T, KT_FC1, 128, B], f16, kind="ExternalInput").ap()
    wfc1_d = nc.dram_tensor("wfc1", [N_IN, H_LOC], f16, kind="ExternalInput").ap()
    wrec_d = nc.dram_tensor("wrec", [N_HID, H_LOC], fp8, kind="ExternalInput").ap()
    wout_d = nc.dram_tensor("wout", [H_LOC, N_OUT], f16, kind="ExternalInput").ap()
    ident_d = nc.dram_tensor("ident", [128, 128], fp8, kind="ExternalInput").ap()
    o_d = nc.dram_tensor("o_part", [T, B, N_OUT], f16, kind="ExternalOutput").ap()

    with tile.TileContext(nc) as tc:
        with (
            tc.tile_pool(name="wpool", bufs=1) as wp,
            tc.tile_pool(name="state", bufs=1) as st,
            tc.tile_pool(name="xtp", bufs=4) as xp,
            tc.tile_pool(name="xprojp", bufs=4) as xpp,
            tc.tile_pool(name="gathp", bufs=2) as gfp,
            tc.tile_pool(name="spktp", bufs=2) as stp,
            tc.tile_pool(name="osbp", bufs=2) as obp,
            tc.tile_pool(name="ps_rec", bufs=2, space="PSUM") as pr,
            tc.tile_pool(name="ps_tr", bufs=2, space="PSUM") as pt,
            tc.tile_pool(name="ps_o", bufs=1, space="PSUM") as po,
            tc.tile_pool(name="ps_x", bufs=1, space="PSUM") as px,
            tc.tile_pool(name="dram_ag", bufs=2, space="DRAM") as dag,
        ):
            # --- weights resident in SBUF for the whole kernel ---
            # load order = first-use order: fc1(0) blocks on wfc1 (+x
            # prefetches below), the readout needs wout only as shadow
            # work, and wrec is first consumed by rec(1)
            wfc1_sb = wp.tile([128, KT_FC1, H_LOC], f16)
            nc.sync.dma_start(
                out=wfc1_sb[:], in_=wfc1_d.rearrange("(k p) n -> p k n", p=128)
            )
            ident_sb = wp.tile([128, 128], fp8)
            nc.sync.dma_start(out=ident_sb[:], in_=ident_d[:])

            # --- persistent state (batch on partitions, local hidden on free) ---
            h_mem = st.tile([128, H_LOC], f32)
            keep = st.tile([128, H_LOC], f32)    # tau * (1 - spk_prev)
            hk = st.tile([128, H_LOC], f32)      # h_mem * keep
            pre = st.tile([128, H_LOC], f32)     # hk + x_proj (pre-REC part)
            thr = st.tile([128, H_LOC], f32)     # 2560*(0.5-pre): spike thr on psum
            spk = st.tile([128, H_LOC], fp8)
            spk_accT = st.tile([128, KT_OUT, B], f16)  # transposed tau-accum

            xt_tiles = {}
            xproj_tiles = {}
            gath_tiles = {}

            def prefetch_xt(t):
                xt_sb = xp.tile([128, KT_FC1, B], f16, name="xt_sb", tag="xt_sb")
                nc.sync.dma_start(
                    out=xt_sb[:], in_=xt_d[t].rearrange("k p b -> p k b")
                )
                xt_tiles[t] = xt_sb

            def fc1(t):
                ps = px.tile([128, H_LOC], f32, name="ps_x_t", tag="psx")
                xt_sb = xt_tiles.pop(t)
                for k in range(KT_FC1):
                    nc.tensor.matmul(
                        ps[:],
                        lhsT=xt_sb[:, k, :],
                        rhs=wfc1_sb[:, k, :],
                        start=(k == 0),
                        stop=(k == KT_FC1 - 1),
                    )
                xs = xpp.tile([128, H_LOC], f32, name="xproj_t", tag="xproj")
                nc.vector.tensor_copy(out=xs[:], in_=ps[:])
                xproj_tiles[t] = xs

            # prologue: input projection for steps 0 and 1
            prefetch_xt(0)
            prefetch_xt(1)
            prefetch_xt(2)
            wout_sb = wp.tile([128, KT_OUT, N_OUT], f16)
            nc.sync.dma_start(
                out=wout_sb[:], in_=wout_d.rearrange("(k p) n -> p k n", p=128)
            )
            wrec_sb = wp.tile([128, KT_REC, H_LOC], fp8)
            nc.sync.dma_start(
                out=wrec_sb[:], in_=wrec_d.rearrange("(k p) n -> p k n", p=128)
            )
            fc1(0)
            fc1(1)

            for t in range(T):
                if t > 0:
                    # pre = hk + x_proj and the spike threshold, both ready
                    # while the REC matmul streams
                    xs = xproj_tiles.pop(t)
                    nc.vector.tensor_tensor(
                        out=pre[:], in0=hk[:], in1=xs[:], op=Alu.add
                    )
                    nc.vector.tensor_scalar(
                        out=thr[:], in0=pre[:],
                        scalar1=-REC_W_SCALE / REC_SCALE,
                        scalar2=REC_W_SCALE * THRESH / REC_SCALE,
                        op0=Alu.mult, op1=Alu.add,
                    )
                    # recurrent matmul over the gathered spikes, fp8
                    # DoubleRow (2 k-tiles per instruction).  First chunk
                    # (k-tiles 0:16) lands on one DMA queue, second on
                    # another, so the matmul starts as soon as chunk A is in.
                    chunks = gath_tiles.pop(t - 1)
                    ps_rec = pr.tile([128, H_LOC], f32, name="ps_rec_t", tag="psrec")
                    for ci, (g, base, n) in enumerate(chunks):
                        for k in range(n):
                            nc.tensor.matmul(
                                ps_rec[:],
                                lhsT=g[:, 2 * k:2 * k + 2, :],
                                rhs=wrec_sb[:, base + 2 * k:base + 2 * k + 2, :],
                                start=(ci == 0 and k == 0),
                                stop=(ci == len(chunks) - 1 and k == n - 1),
                                perf_mode=DR,
                            )
                    # spike decision straight off PSUM:
                    # psum >= 2560*(0.5-pre)  <=>  0.1*rec + pre >= 0.5
                    nc.vector.tensor_tensor(
                        out=spk[:], in0=ps_rec[:], in1=thr[:], op=Alu.is_ge
                    )
                else:
                    xs = xproj_tiles.pop(0)
                    nc.vector.tensor_copy(out=h_mem[:], in_=xs[:])
                    nc.vector.tensor_scalar(
                        out=spk[:], in0=h_mem[:], scalar1=THRESH, scalar2=None,
                        op0=Alu.is_ge,
                    )

                # transpose local spikes: [b, h_loc] -> [h_low, j, b];
                # cast to fp8, bounce out, single AllGather, land in 2
                # parallel chunks.
                # fp8 PE transpose requires output element step 2: write
                # plane 0 of a [..., 2]-strided PSUM tile
                ps_tr2 = pt.tile([128, KT_OUT, B, 2], fp8, name="ps_tr_t", tag="pstr")
                ps_tr = ps_tr2[:, :, :, 0]
                for j in range(KT_OUT):
                    nc.tensor.transpose(
                        ps_tr2[:, j, :, 0], spk[:, j * 128:(j + 1) * 128],
                        ident_sb[:],
                    )
                if t < T - 1:
                    spk8 = stp.tile([128, KT_OUT, B], fp8, name="spk8_t", tag="spk8")
                    nc.vector.tensor_copy(out=spk8[:], in_=ps_tr)
                    ag_in = dag.tile([H_LOC, B], fp8, name="ag_in_t", tag="agin")
                    ag_v = ag_in.rearrange("(j p) b -> p j b", p=128)
                    # bounce as two parallel 32KB halves on separate queues
                    nc.sync.dma_start(out=ag_v[:, 0:2, :], in_=spk8[:, 0:2, :])
                    nc.scalar.dma_start(out=ag_v[:, 2:4, :], in_=spk8[:, 2:4, :])
                    ag_out = dag.tile(
                        [N_HID, B], fp8, addr_space="Shared",
                        name="ag_out_t", tag="agout",
                    )
                    nc.gpsimd.collective_compute(
                        "AllGather",
                        Alu.bypass,
                        replica_groups=[list(range(N_CORES))],
                        ins=[ag_in.opt()],
                        outs=[ag_out.opt()],
                    )
                    g_view = ag_out.rearrange("(k p) b -> p k b", p=128)
                    # progressive landing: small first chunk so the rec
                    # matmul starts ~1.5us after AG completion and streams
                    # the rest as the 3 DMA queues deliver it
                    spans = [(0, 2), (2, 12), (12, 20), (20, 26), (26, 32)]
                    engines = [nc.sync, nc.scalar, nc.gpsimd, nc.sync,
                               nc.scalar]
                    gs = []
                    for ci, ((lo, hi), eng) in enumerate(zip(spans, engines)):
                        g = gfp.tile(
                            [128, hi - lo, B], fp8,
                            name=f"g8_{ci}_t", tag=f"g8_{ci}",
                        )
                        eng.dma_start(out=g[:], in_=g_view[:, lo:hi, :])
                        gs.append((g, lo, (hi - lo) // 2))
                    gath_tiles[t] = gs

                # membrane update + reset mask, off the critical path
                # (runs in the AllGather shadow)
                if t > 0:
                    nc.vector.scalar_tensor_tensor(
                        out=h_mem[:], in0=ps_rec[:],
                        scalar=REC_SCALE / REC_W_SCALE, in1=pre[:],
                        op0=Alu.mult, op1=Alu.add,
                    )
                nc.vector.tensor_scalar(
                    out=keep[:], in0=spk[:], scalar1=-TAU, scalar2=TAU,
                    op0=Alu.mult, op1=Alu.add,
                )
                nc.vector.tensor_tensor(
                    out=hk[:], in0=h_mem[:], in1=keep[:], op=Alu.mult
                )

                # tau-accumulated (transposed) spikes for the readout,
                # read straight from the transpose PSUM
                if t == 0:
                    nc.vector.tensor_copy(out=spk_accT[:], in_=ps_tr)
                else:
                    nc.vector.scalar_tensor_tensor(
                        out=spk_accT[:], in0=spk_accT[:], scalar=TAU, in1=ps_tr,
                        op0=Alu.mult, op1=Alu.add,
                    )

                # readout partial: o_t[b, :] = spk_acc_t[:, h_loc] @ W_out[h_loc, :]
                ps_o = po.tile([128, N_OUT], f32, name="ps_o_t", tag="pso")
                for n in range(N_OUT // 512):
                    for k in range(KT_OUT):
                        nc.tensor.matmul(
                            ps_o[:, n * 512:(n + 1) * 512],
                            lhsT=spk_accT[:, k, :],
                            rhs=wout_sb[:, k, n * 512:(n + 1) * 512],
                            start=(k == 0),
                            stop=(k == KT_OUT - 1),
                        )
                o_sb = obp.tile([128, N_OUT], f16, name="o_sb_t", tag="osb")
                nc.vector.tensor_copy(out=o_sb[:], in_=ps_o[:])
                nc.sync.dma_start(out=o_d[t], in_=o_sb[:])

                # pipelined input projection for step t+2
                if t + 2 < T:
                    fc1(t + 2)
                if t + 3 < T:
                    prefetch_xt(t + 3)

    nc.compile()
    return nc


def _get_compiled():
    if "nc" not in _cache:
        _cache["nc"] = _build()
    return _cache["nc"]


def _make_in_maps(x, W_fc1, W_rec, W_out):
    import ml_dtypes

    x = np.asarray(x, dtype=np.float32)
    W_fc1 = np.asarray(W_fc1, dtype=np.float32)
    W_rec = np.asarray(W_rec, dtype=np.float32)
    W_out = np.asarray(W_out, dtype=np.float32)

    # [B, T, N] -> [T, N, B] -> [T, k, p, B]
    xt = (
        np.ascontiguousarray(x.transpose(1, 2, 0))
        .reshape(T, N_IN // 128, 128, B)
        .astype(np.float16)
    )
    ident = np.eye(128, dtype=ml_dtypes.float8_e4m3fn)

    in_maps = []
    for c in range(N_CORES):
        lo, hi = c * H_LOC, (c + 1) * H_LOC
        in_maps.append(
            {
                "xt": xt,
                "wfc1": np.ascontiguousarray(W_fc1[:, lo:hi]).astype(np.float16),
                "wrec": np.ascontiguousarray(
                    W_rec[:, lo:hi] * REC_W_SCALE
                ).astype(ml_dtypes.float8_e4m3fn),
                "wout": np.ascontiguousarray(W_out[lo:hi, :]).astype(np.float16),
                "ident": ident,
            }
        )
    return in_maps


def _combine(results):
    o = np.zeros((T, B, N_OUT), dtype=np.float64)
    for c in range(N_CORES):
        o += results[c]["o_part"]
    return np.ascontiguousarray(o.transpose(1, 0, 2)).astype(np.float32)


def kernel(x, W_fc1, W_rec, W_out):
    from concourse.bass_utils import run_bass_kernel_spmd

    nc = _get_compiled()
    in_maps = _make_in_maps(x, W_fc1, W_rec, W_out)
    res = run_bass_kernel_spmd(nc, in_maps, core_ids=list(range(N_CORES)))
    return _combine(res.results)


# revision 21
# speedup vs baseline: 1.0176x; 1.0176x over previous
"""Trainium2 Bass kernel for the RSNN (recurrent spiking NN) problem.

Strategy: model-parallel over the hidden dimension across 8 NeuronCores
(512 hidden units per core), full batch (128) on every core.  Per
timestep each core computes the recurrent matmul for its hidden shard
([128x4096] @ [4096x512]), derives its shard's spikes, and AllGathers
the (PE-transposed) spikes as fp8 so every core has the full spike
vector for the next step.  The input projection (x @ W_fc1) is
pipelined two steps ahead and the readout (spk_acc @ W_out) is computed
per-step from the tau-accumulated spikes - both fill the PE while the
AllGather is in flight.  Partial readout sums (over hidden shards) are
combined on the host.

  o_mem_t = sum_{s<=t} tau^{t-s} spk_s @ W_out = (spk_acc_t) @ W_out
  spk_acc_t = tau * spk_acc_{t-1} + spk_t

The recurrent matmul runs in fp8 DoubleRow mode (2 contraction tiles
per instruction, 2x fp16 throughput): the gathered spikes are exactly
representable in fp8, and W_rec is pre-scaled by 256 (its entries live
in [0, 1/64] - raw e4m3 would put them all in the subnormal range) so
quantization error is ~2^-4 relative.  The spike decision compensates
the scale: psum >= 2560*(0.5 - pre)  <=>  0.1*rec + pre >= 0.5.
One AllGather per step (two half-gathers serialize completely in the
collectives firmware, each paying its own ~7us floor), landed in two
chunks on two DMA queues so the recurrent matmul can start on the
first 16 contraction tiles while the rest lands.
"""

import sys

sys.path.insert(0, "/opt/trn_rl_repo")

import numpy as np

B, T = 128, 64
N_IN, N_HID, N_OUT = 1024, 4096, 1024
TAU = 0.9
THRESH = 0.5
REC_SCALE = 0.1
N_CORES = 8
H_LOC = N_HID // N_CORES  # 512
REC_W_SCALE = 256.0  # fp8 pre-scale for W_rec

_cache = {}


def _build():
    import concourse.bacc as bacc
    import concourse.tile as tile
    from concourse import mybir

    f32 = mybir.dt.float32
    f16 = mybir.dt.float16
    fp8 = mybir.dt.float8e4
    Alu = mybir.AluOpType
    DR = mybir.MatmulPerfMode.DoubleRow

    KT_REC = N_HID // 128  # 32 contraction tiles for the recurrent matmul
    KT_FC1 = N_IN // 128   # 8
    KT_OUT = H_LOC // 128  # 4
    KT_HALF = KT_REC // 2  # 16 k-tiles per landing chunk

    nc = bacc.Bacc(
        "TRN2",
        target_bir_lowering=False,
        debug=False,
        enable_asserts=True,
        num_devices=N_CORES,
    )

    # x transposed+tiled on host: [T, k, p, B] with n_in = 128*k + p
    xt_d = nc.dram_tensor("xt", [T, KT_FC1, 128, B], f16, kind="ExternalInput").ap()
    wfc1_d = nc.dram_tensor("wfc1", [N_IN, H_LOC], f16, kind="ExternalInput").ap()
    wrec_d = nc.dram_tensor("wrec", [N_HID, H_LOC], fp8, kind="ExternalInput").ap()
    wout_d = nc.dram_tensor("wout", [H_LOC, N_OUT], f16, kind="ExternalInput").ap()
    ident_d = nc.dram_tensor("ident", [128, 128], fp8, kind="ExternalInput").ap()
    o_d = nc.dram_tensor("o_part", [T, B, N_OUT], f16, kind="ExternalOutput").ap()

    with tile.TileContext(nc) as tc:
        with (
            tc.tile_pool(name="wpool", bufs=1) as wp,
            tc.tile_pool(name="state", bufs=1) as st,
            tc.tile_pool(name="xtp", bufs=4) as xp,
            tc.tile_pool(name="xprojp", bufs=4) as xpp,
            tc.tile_pool(name="gathp", bufs=2) as gfp,
            tc.tile_pool(name="spktp", bufs=2) as stp,
            tc.tile_pool(name="osbp", bufs=2) as obp,
            tc.tile_pool(name="ps_rec", bufs=2, space="PSUM") as pr,
            tc.tile_pool(name="ps_tr", bufs=2, space="PSUM") as pt,
            tc.tile_pool(name="ps_o", bufs=1, space="PSUM") as po,
            tc.tile_pool(name="ps_x", bufs=1, space="PSUM") as px,
            tc.tile_pool(name="dram_ag", bufs=2, space="DRAM") as dag,
        ):
            # --- weights resident in SBUF for the whole kernel ---
            # load order = first-use order: fc1(0) blocks on wfc1 (+x
            # prefetches below), the readout needs wout only as shadow
            # work, and wrec is first consumed by rec(1)
            wfc1_sb = wp.tile([128, KT_FC1, H_LOC], f16)
            nc.sync.dma_start(
                out=wfc1_sb[:], in_=wfc1_d.rearrange("(k p) n -> p k n", p=128)
            )
            ident_sb = wp.tile([128, 128], fp8)
            nc.sync.dma_start(out=ident_sb[:], in_=ident_d[:])

            # --- persistent state (batch on partitions, local hidden on free) ---
            h_mem = st.tile([128, H_LOC], f32)
            keep = st.tile([128, H_LOC], f32)    # tau * (1 - spk_prev)
            hk = st.tile([128, H_LOC], f32)      # h_mem * keep
            pre = st.tile([128, H_LOC], f32)     # hk + x_proj (pre-REC part)
            thr = st.tile([128, H_LOC], f32)     # 2560*(0.5-pre): spike thr on psum
            spk = st.tile([128, H_LOC], fp8)
            spk_accT = st.tile([128, KT_OUT, B], f16)  # transposed tau-accum

            xt_tiles = {}
            xproj_tiles = {}
            gath_tiles = {}

            def prefetch_xt(t):
                xt_sb = xp.tile([128, KT_FC1, B], f16, name="xt_sb", tag="xt_sb")
                nc.sync.dma_start(
                    out=xt_sb[:], in_=xt_d[t].rearrange("k p b -> p k b")
                )
                xt_tiles[t] = xt_sb

            def fc1(t):
                ps = px.tile([128, H_LOC], f32, name="ps_x_t", tag="psx")
                xt_sb = xt_tiles.pop(t)
                for k in range(KT_FC1):
                    nc.tensor.matmul(
                        ps[:],
                        lhsT=xt_sb[:, k, :],
                        rhs=wfc1_sb[:, k, :],
                        start=(k == 0),
                        stop=(k == KT_FC1 - 1),
                    )
                xs = xpp.tile([128, H_LOC], f32, name="xproj_t", tag="xproj")
                nc.vector.tensor_copy(out=xs[:], in_=ps[:])
                xproj_tiles[t] = xs

            # prologue: input projection for steps 0 and 1
            prefetch_xt(0)
            prefetch_xt(1)
            prefetch_xt(2)
            wout_sb = wp.tile([128, KT_OUT, N_OUT], f16)
            nc.sync.dma_start(
                out=wout_sb[:], in_=wout_d.rearrange("(k p) n -> p k n", p=128)
            )
            wrec_sb = wp.tile([128, KT_REC, H_LOC], fp8)
            nc.sync.dma_start(
                out=wrec_sb[:], in_=wrec_d.rearrange("(k p) n -> p k n", p=128)
            )
            fc1(0)
            fc1(1)

            for t in range(T):
                if t > 0:
                    # pre = hk + x_proj and the spike threshold, both ready
                    # while the REC matmul streams
                    xs = xproj_tiles.pop(t)
                    nc.vector.tensor_tensor(
                        out=pre[:], in0=hk[:], in1=xs[:], op=Alu.add
                    )
                    nc.vector.tensor_scalar(
                        out=thr[:], in0=pre[:],
                        scalar1=-REC_W_SCALE / REC_SCALE,
                        scalar2=REC_W_SCALE * THRESH / REC_SCALE,
                        op0=Alu.mult, op1=Alu.add,
                    )
                    # recurrent matmul over the gathered spikes, fp8
                    # DoubleRow (2 k-tiles per instruction).  First chunk
                    # (k-tiles 0:16) lands on one DMA queue, second on
                    # another, so the matmul starts as soon as chunk A is in.
                    chunks = gath_tiles.pop(t - 1)
                    ps_rec = pr.tile([128, H_LOC], f32, name="ps_rec_t", tag="psrec")
                    for ci, (g, base, n) in enumerate(chunks):
                        for k in range(n):
                            nc.tensor.matmul(
                                ps_rec[:],
                                lhsT=g[:, 2 * k:2 * k + 2, :],
                                rhs=wrec_sb[:, base + 2 * k:base + 2 * k + 2, :],
                                start=(ci == 0 and k == 0),
                                stop=(ci == len(chunks) - 1 and k == n - 1),
                                perf_mode=DR,
                            )
                    # spike decision straight off PSUM:
                    # psum >= 2560*(0.5-pre)  <=>  0.1*rec + pre >= 0.5
                    nc.vector.tensor_tensor(
                        out=spk[:], in0=ps_rec[:], in1=thr[:], op=Alu.is_ge
                    )
                else:
                    xs = xproj_tiles.pop(0)
                    nc.vector.tensor_copy(out=h_mem[:], in_=xs[:])
                    nc.vector.tensor_scalar(
                        out=spk[:], in0=h_mem[:], scalar1=THRESH, scalar2=None,
                        op0=Alu.is_ge,
                    )

                # transpose local spikes: [b, h_loc] -> [h_low, j, b];
                # cast to fp8, bounce out, single AllGather, land in 2
                # parallel chunks.
                # fp8 PE transpose requires output element step 2: write
                # plane 0 of a [..., 2]-strided PSUM tile
                ps_tr2 = pt.tile([128, KT_OUT, B, 2], fp8, name="ps_tr_t", tag="pstr")
                ps_tr = ps_tr2[:, :, :, 0]
                for j in range(KT_OUT):
                    nc.tensor.transpose(
                        ps_tr2[:, j, :, 0], spk[:, j * 128:(j + 1) * 128],
                        ident_sb[:],
                    )
                if t < T - 1:
                    spk8 = stp.tile([128, KT_OUT, B], fp8, name="spk8_t", tag="spk8")
                    nc.vector.tensor_copy(out=spk8[:], in_=ps_tr)
                    ag_in = dag.tile([H_LOC, B], fp8, name="ag_in_t", tag="agin")
                    nc.scalar.dma_start(
                        out=ag_in.rearrange("(j p) b -> p j b", p=128),
                        in_=spk8[:],
                    )
                    ag_out = dag.tile(
                        [N_HID, B], fp8, addr_space="Shared",
                        name="ag_out_t", tag="agout",
                    )
                    nc.gpsimd.collective_compute(
                        "AllGather",
                        Alu.bypass,
                        replica_groups=[list(range(N_CORES))],
                        ins=[ag_in.opt()],
                        outs=[ag_out.opt()],
                    )
                    g_view = ag_out.rearrange("(k p) b -> p k b", p=128)
                    # progressive landing: small first chunk so the rec
                    # matmul starts ~1.5us after AG completion and streams
                    # the rest as the 3 DMA queues deliver it
                    spans = [(0, 2), (2, 12), (12, 20), (20, 26), (26, 32)]
                    engines = [nc.sync, nc.scalar, nc.gpsimd, nc.sync,
                               nc.scalar]
                    gs = []
                    for ci, ((lo, hi), eng) in enumerate(zip(spans, engines)):
                        g = gfp.tile(
                            [128, hi - lo, B], fp8,
                            name=f"g8_{ci}_t", tag=f"g8_{ci}",
                        )
                        eng.dma_start(out=g[:], in_=g_view[:, lo:hi, :])
                        gs.append((g, lo, (hi - lo) // 2))
                    gath_tiles[t] = gs

                # membrane update + reset mask, off the critical path
                # (runs in the AllGather shadow)
                if t > 0:
                    nc.vector.scalar_tensor_tensor(
                        out=h_mem[:], in0=ps_rec[:],
                        scalar=REC_SCALE / REC_W_SCALE, in1=pre[:],
                        op0=Alu.mult, op1=Alu.add,
                    )
                nc.vector.tensor_scalar(
                    out=keep[:], in0=spk[:], scalar1=-TAU, scalar2=TAU,
                    op0=Alu.mult, op1=Alu.add,
                )
                nc.vector.tensor_tensor(
                    out=hk[:], in0=h_mem[:], in1=keep[:], op=Alu.mult
                )

                # tau-accumulated (transposed) spikes for the readout,
                # read straight from the transpose PSUM
                if t == 0:
                    nc.vector.tensor_copy(out=spk_accT[:], in_=ps_tr)
                else:
                    nc.vector.scalar_tensor_tensor(
                        out=spk_accT[:], in0=spk_accT[:], scalar=TAU, in1=ps_tr,
                        op0=Alu.mult, op1=Alu.add,
                    )

                # readout partial: o_t[b, :] = spk_acc_t[:, h_loc] @ W_out[h_loc, :]
                ps_o = po.tile([128, N_OUT], f32, name="ps_o_t", tag="pso")
                for n in range(N_OUT // 512):
                    for k in range(KT_OUT):
                        nc.tensor.matmul(
                            ps_o[:, n * 512:(n + 1) * 512],
                            lhsT=spk_accT[:, k, :],
                            rhs=wout_sb[:, k, n * 512:(n + 1) * 512],
                            start=(k == 0),
                            stop=(k == KT_OUT - 1),
                        )
                o_sb = obp.tile([128, N_OUT], f16, name="o_sb_t", tag="osb")
                nc.vector.tensor_copy(out=o_sb[:], in_=ps_o[:])
                nc.gpsimd.dma_start(out=o_d[t], in_=o_sb[:])

                # pipelined input projection for step t+2
                if t + 2 < T:
                    fc1(t + 2)
                if t + 3 < T:
                    prefetch_xt(t + 3)

    nc.compile()
    return nc


def _get_compiled():
    if "nc" not in _cache:
        _cache["nc"] = _build()
    return _cache["nc"]


def _make_in_maps(x, W_fc1, W_rec, W_out):
    import ml_dtypes

    x = np.asarray(x, dtype=np.float32)
    W_fc1 = np.asarray(W_fc1, dtype=np.float32)
    W_rec = np.asarray(W_rec, dtype=np.float32)
    W_out = np.asarray(W_out, dtype=np.float32)

    # [B, T, N] -> [T, N, B] -> [T, k, p, B]
    xt = (
        np.ascontiguousarray(x.transpose(1, 2, 0))
        .reshape(T, N_IN // 128, 128, B)
        .astype(np.float16)
    )
    ident = np.eye(128, dtype=ml_dtypes.float8_e4m3fn)

    in_maps = []
    for c in range(N_CORES):
        lo, hi = c * H_LOC, (c + 1) * H_LOC
        in_maps.append(
            {
                "xt": xt,
                "wfc1": np.ascontiguousarray(W_fc1[:, lo:hi]).astype(np.float16),
                "wrec": np.ascontiguousarray(
                    W_rec[:, lo:hi] * REC_W_SCALE
                ).astype(ml_dtypes.float8_e4m3fn),
                "wout": np.ascontiguousarray(W_out[lo:hi, :]).astype(np.float16),
                "ident": ident,
            }
        )
    return in_maps


def _combine(results):
    o = np.zeros((T, B, N_OUT), dtype=np.float64)
    for c in range(N_CORES):
        o += results[c]["o_part"]
    return np.ascontiguousarray(o.transpose(1, 0, 2)).astype(np.float32)


def kernel(x, W_fc1, W_rec, W_out):
    from concourse.bass_utils import run_bass_kernel_spmd

    nc = _get_compiled()
    in_maps = _make_in_maps(x, W_fc1, W_rec, W_out)
    res = run_bass_kernel_spmd(nc, in_maps, core_ids=list(range(N_CORES)))
    return _combine(res.results)
